# revision 63
# baseline (speedup 1.0000x reference)
"""Trainium2 Bass kernel for CrossModalFusion (B=4, C=64, H=W=64, N=4096).

Reference computation (per sample b, with x reshaped to [C, N]):
    q = wq @ xo + bq          [8, N]
    k = wk @ xs + bk          [8, N]
    v = wv @ xs + bv          [64, N]
    S[n, m]  = q[:, n] . k[:, m]
    attn     = softmax_m(S)
    out      = gamma * (v @ attn^T) + x_opt

Sharding: 8 cores = 4 batch samples x 2 halves of the query (n) axis.
Each core computes output rows [64, 2048] for its (sample, n-half); no
cross-core communication is needed.

Per-core dataflow:
  - TRN2 PE observation this kernel is built around: a single matmul
    instruction streams moving columns at ~1.2 Gcol/s regardless of dtype,
    but TWO matmuls in disjoint row tiles run concurrently (~2x), and fp8
    DoubleRow processes 2 contraction k-tiles per pass (~1.5x).
  - stacked-halves layout: inputs arrive with the free axis split across
    partitions 0-63 / 64-127, so every q/k/v projection matmul has a
    concurrent partner in the opposite row tile, and k lands at both row
    groups without replication (k rows 0-7 = m 0-2047, rows 64-71 =
    m 2048-4095).  Biases are added in the PSUM->SBUF casts (split across
    the ACT and DVE engines); gamma is folded into wv on the host (the
    denominator column stays 1, so gamma cancels in the divide).
  - scores are computed TRANSPOSED (S^T[m, n]) in bf16: wave w scores
    m-block w (rows 0-7) concurrently with m-block 16+w (rows 64-71) into
    one fp32 PSUM tile; exp runs on ACT reading PSUM and writing fp8e4
    SBUF directly (bias -1.5 keeps e^s inside fp8 range; it cancels in
    the softmax ratio).  No max-subtraction is needed: scores are O(3).
  - the attention*V accumulation is ONE fp8 DoubleRow matmul per wave
    (k-tile pair = blocks w and 16+w, stationary v^T padded to 80 cols so
    the k-tile step is 16-aligned).  v^T's column 64 is the all-ones
    denominator column, so sum_m exp(S[n, m]) accumulates for free.
  - the wave loop runs at the ACT exp floor (~1.11us per 2 m-blocks);
    AV lags two waves (one at tile end) so the PE never waits on exp.
  - per n-tile of 512: normalize via reciprocal_approx_fast (custom DVE
    op, needs an SBUF source), a rank-1 ones broadcast matmul, and two
    DVE element-wise ops.  The apply for tile t is emitted at wave 6 of
    tile t+1, after its reciprocal has finished, so the in-order PE queue
    never blocks on it.
  - residual x_opt is DMA'd separately in fp32 (off the critical path) so
    the gamma=0 output is exact (attention term is exactly zero).
"""

import os
import sys

import numpy as np

for _p in ("/opt/trn_rl_repo", "/root/.axon_site/_ro/trn_rl_repo"):
    if os.path.isdir(_p) and _p not in sys.path:
        sys.path.insert(0, _p)

import ml_dtypes

import concourse.bass as bass
import concourse.mybir as mybir
import concourse.tile as tile
from concourse import bacc
from concourse.bass_utils import run_bass_kernel_spmd

F32 = mybir.dt.float32
F32R = mybir.dt.float32r
BF16 = mybir.dt.bfloat16
FP8 = mybir.dt.float8e4
AF = mybir.ActivationFunctionType
NP_BF16 = np.dtype(ml_dtypes.bfloat16)
EXP_BIAS = -1.5  # exp(s + EXP_BIAS): keeps e_t within fp8e4 range; the
                 # common factor cancels between numerator and denominator

B, C, HH, WW = 4, 64, 64, 64
N = HH * WW            # 4096 key/query positions
D = 8                  # q/k channel count
CA = C + 1             # augmented channel dim (ones row / denominator row)
VW = 80                # padded v^T block width (DoubleRow k-tile step %16==0)
WCOLS = D + D + VW     # packed weight buffer width (wq | wk | wv')
NCORES = 8
NL = N // 2            # query rows per core
NT = 512               # n-tile (PSUM bank width in fp32)
MB = 128               # m-block (PE partition width)
N_NT = NL // NT        # 4 n-tiles per core
N_MB = N // MB         # 32 m-blocks
WAVE = 2               # m-blocks exp'd per ACT instruction


def build_program(repeat: int = 1) -> bass.Bass:
    # Bacc (not raw Bass): its compile() pass splits multi-semaphore waits
    # and moves matmul waits onto LDWEIGHTS, which this walrus build requires.
    nc = bacc.Bacc("TRN2", target_bir_lowering=False, num_devices=NCORES)
    # stacked-halves layout: partitions 0-63 hold one half of the data,
    # 64-127 the other, so every preamble matmul has a concurrent partner
    # in the opposite PE row tile (2x aggregate column throughput).
    xo_d = nc.declare_dram_parameter("xo_st", [MB, NL // 2], BF16, isOutput=False)
    xs_d = nc.declare_dram_parameter("xs_st", [MB, N // 2], BF16, isOutput=False)
    xof_d = nc.declare_dram_parameter("xof", [C, NL], F32, isOutput=False)
    w_d = nc.declare_dram_parameter("w2", [MB, WCOLS], BF16, isOutput=False)
    bp_d = nc.declare_dram_parameter("bpack", [64 + D, 2], F32, isOutput=False)
    bv_d = nc.declare_dram_parameter("bvg4", [MB, 4 * VW], F32, isOutput=False)
    out_d = nc.declare_dram_parameter("out", [C, NL], F32, isOutput=True)

    with tile.TileContext(nc) as tc:
      for _rep in range(repeat):
        with tc.tile_pool(name="const", bufs=1) as cp:
            # --- input DMAs, spread across per-engine DGE queues ---
            w_sb = cp.tile([MB, WCOLS], BF16)
            nc.gpsimd.dma_start(w_sb[:], w_d[:])
            xo_sb = cp.tile([MB, NL // 2], BF16)
            nc.sync.dma_start(xo_sb[:], xo_d[:])
            xs_sb = cp.tile([MB, N // 2], BF16)

            def load_xs(j):
                nc.gpsimd.dma_start(
                    xs_sb[:, j * 1024 : (j + 1) * 1024],
                    xs_d[:, j * 1024 : (j + 1) * 1024],
                )

            load_xs(0)
            bp_sb = cp.tile([64 + D, 2], F32)
            nc.gpsimd.dma_start(bp_sb[:], bp_d[:])
            bvg_sb = cp.tile([MB, 4 * VW], F32)
            nc.gpsimd.dma_start(bvg_sb[:], bv_d[:])
            xof_sb = cp.tile([C, NL], F32)
            ones_sb = cp.tile([1, C], BF16)
            nc.vector.memset(ones_sb[:], 1.0)
            ebias_sb = cp.tile([MB, 1], F32)
            nc.vector.memset(ebias_sb[:], EXP_BIAS)
            wqA, wqB = w_sb[0:64, 0:D], w_sb[64:MB, 0:D]
            wkA, wkB = w_sb[0:64, D : 2 * D], w_sb[64:MB, D : 2 * D]
            wvA, wvB = w_sb[0:64, 2 * D :], w_sb[64:MB, 2 * D :]

            # per-chunk tiles (not one big tile): the tile framework's
            # waits collapse to a tile's LAST writer, so fine-grained tiles
            # keep the first waves from waiting on late preamble casts.
            # qt[nt] serves n-tile nt; kt[j]/vt[g] serve 4 waves each.
            qt = [cp.tile([64 + D, NT], BF16, name=f"qt{i}") for i in range(4)]
            kt = [cp.tile([64 + D, NT], BF16, name=f"kt{i}") for i in range(4)]
            # v^T blocks in fp8e4: feeds the DoubleRow attention*V matmul
            vt = [cp.tile([MB, 8, VW], FP8, name=f"vt{i}") for i in range(4)]

            with tc.tile_pool(name="pre_ps", bufs=2, space="PSUM") as pp:
                def emit_q(j):
                    # halves A/B run concurrently in row tiles 0/64
                    qp = pp.tile([MB, NT], F32, tag="qk_ps")
                    nc.tensor.matmul(
                        qp[0:D, :], wqA, xo_sb[0:64, j * NT : (j + 1) * NT],
                        start=True, stop=True,
                    )
                    nc.tensor.matmul(
                        qp[64 : 64 + D, :], wqB,
                        xo_sb[64:MB, j * NT : (j + 1) * NT],
                        start=True, stop=True,
                    )
                    slA = qt[j][0:D, :]
                    slB = qt[j + 2][64 : 64 + D, :]
                    # bias add fused into the PSUM->bf16 casts.  Only the
                    # first-wave-critical casts go on ACT (the ACT queue is
                    # in order, and exps must not sit behind late casts).
                    if j == 0:
                        nc.scalar.activation(
                            slA, qp[0:D, :], AF.Identity, bias=bp_sb[0:D, 0:1]
                        )
                    else:
                        nc.vector.tensor_scalar_add(
                            slA, qp[0:D, :], bp_sb[0:D, 0:1]
                        )
                    nc.vector.tensor_scalar_add(
                        slB, qp[64 : 64 + D, :], bp_sb[64 : 64 + D, 0:1]
                    )
                    # partition-offset replicas: engines are lane-aligned,
                    # so these must be DMAs
                    nc.gpsimd.dma_start(qt[j][64 : 64 + D, :], slA)
                    nc.gpsimd.dma_start(qt[j + 2][0:D, :], slB)

                def emit_k(j):
                    kp = pp.tile([MB, NT], F32, tag="qk_ps")
                    nc.tensor.matmul(
                        kp[0:D, :], wkA, xs_sb[0:64, j * NT : (j + 1) * NT],
                        start=True, stop=True,
                    )
                    nc.tensor.matmul(
                        kp[64 : 64 + D, :], wkB,
                        xs_sb[64:MB, j * NT : (j + 1) * NT],
                        start=True, stop=True,
                    )
                    nc.vector.tensor_scalar_add(
                        kt[j][0:D, :], kp[0:D, :], bp_sb[0:D, 1:2]
                    )
                    if j == 0:
                        nc.scalar.activation(
                            kt[j][64 : 64 + D, :], kp[64 : 64 + D, :],
                            AF.Identity, bias=bp_sb[64 : 64 + D, 1:2],
                        )
                    else:
                        nc.vector.tensor_scalar_add(
                            kt[j][64 : 64 + D, :], kp[64 : 64 + D, :],
                            bp_sb[64 : 64 + D, 1:2],
                        )

                def emit_v(g):
                    # 4 m-block pairs per call; halves A/B concurrent; the
                    # bias row (bv*gamma | denominator 1) is added in the
                    # PSUM->fp8 cast
                    vpA = pp.tile([MB, 4 * VW], F32, tag="vpa_ps")
                    vpB = pp.tile([MB, 4 * VW], F32, tag="vpb_ps")
                    for t in range(4):
                        a = 4 * g + t
                        nc.tensor.matmul(
                            vpA[:, t * VW : (t + 1) * VW],
                            xs_sb[0:64, a * MB : (a + 1) * MB], wvA,
                            start=True, stop=True,
                        )
                        nc.tensor.matmul(
                            vpB[:, t * VW : (t + 1) * VW],
                            xs_sb[64:MB, a * MB : (a + 1) * MB], wvB,
                            start=True, stop=True,
                        )
                    nc.vector.tensor_add(vt[g][:, 0:4, :], vpA[:], bvg_sb[:])
                    nc.vector.tensor_add(vt[g][:, 4:8, :], vpB[:], bvg_sb[:])

                # ordered by first use in the wave loop
                emit_q(0); emit_k(0); emit_v(0)
                load_xs(1)
                emit_q(1); emit_k(1); emit_v(1)
                emit_k(2); emit_v(2)
                emit_k(3); emit_v(3)

            # residual; needed late (first normalize), queued after replicas
            for j in range(2):
                nc.gpsimd.dma_start(
                    xof_sb[:, j * 1024 : (j + 1) * 1024],
                    xof_d[:, j * 1024 : (j + 1) * 1024],
                )

            with (
                tc.tile_pool(name="st_ps", bufs=2, space="PSUM") as st_pool,
                tc.tile_pool(name="av_ps", bufs=2, space="PSUM") as av_pool,
                tc.tile_pool(name="bc_ps", bufs=1, space="PSUM") as bc_pool,
                tc.tile_pool(name="e_sb", bufs=6) as e_pool,
                tc.tile_pool(name="o_sb", bufs=2) as o_pool,
                tc.tile_pool(name="sm_sb", bufs=2) as sm_pool,
            ):
                def norm_recip(av):
                    # softmax denominator -> reciprocal (row 64 of av).
                    # reciprocal_approx_fast (custom DVE op, ~51 ULP) is ~5x
                    # faster than the iterative divide; it MUST read SBUF (a
                    # PSUM source silently computes garbage), so stage the
                    # denominator row out of PSUM first.
                    ds = sm_pool.tile([1, NT], F32, tag="ds")
                    nc.vector.tensor_copy(ds[:], av[C:CA, :])
                    r = sm_pool.tile([1, NT], F32, tag="r")
                    nc.vector.reciprocal_approx_fast(out=r[:], in_=ds[:])
                    # bf16 copy feeds the broadcast matmul (bf16 is ~2x
                    # faster than the fp32 LOW/HIGH pair on the drain path)
                    rb = sm_pool.tile([1, NT], BF16, tag="rb")
                    nc.vector.tensor_copy(rb[:], r[:])
                    return rb

                def norm_apply(av, rb, nt):
                    n0, n1 = nt * NT, (nt + 1) * NT
                    # broadcast 1/denom across the 64 channel partitions via
                    # a rank-1 ones matmul
                    bc = bc_pool.tile([C, NT], F32)
                    nc.tensor.matmul(bc[:], ones_sb[:], rb[:], start=True, stop=True)
                    bcs = o_pool.tile([C, NT], F32, tag="bcs")
                    nc.vector.tensor_copy(bcs[:], bc[:])
                    om = o_pool.tile([C, NT], F32, tag="om")
                    nc.vector.tensor_mul(om[:], av[0:C, :], bcs[:])
                    o = o_pool.tile([C, NT], F32, tag="o")
                    nc.vector.tensor_add(o[:], om[:], xof_sb[:, n0:n1])
                    nc.sync.dma_start(out_d[:, n0:n1], o[:])

                prev = None  # (av, r, nt) awaiting apply
                for nt in range(N_NT):
                    n0, n1 = nt * NT, (nt + 1) * NT
                    av = av_pool.tile([CA, NT], F32)

                    def emit_av(e_t, w, av=av):
                        # fp8 DoubleRow: one matmul accumulates BOTH m-blocks
                        # of the wave (k-tile pair = blocks w and 16+w = dims
                        # w%4 and 4+w%4 of vt[w//4], stride 4 blocks)
                        a = vt[w // 4][:, (w % 4) : (w % 4) + 2, 0:CA]
                        lhsT = bass.AP(
                            tensor=a.tensor, offset=a.offset,
                            ap=[list(a.ap[0]), [4 * VW, 2], [1, CA]],
                        )
                        nc.tensor.matmul(
                            av[:],
                            lhsT,
                            e_t[:],
                            perf_mode=mybir.MatmulPerfMode.DoubleRow,
                            start=(w == 0),
                            stop=(w == N_MB // WAVE - 1),
                        )

                    # S^T matmuls + exp, with the AV accumulation lagging two
                    # waves so the PE never stalls waiting on the current exp.
                    # Wave w scores m-block w (rows 0-7) and m-block 16+w
                    # (rows 64-71) -- both live at column w*128 of k_rep.
                    pend = []
                    for w in range(N_MB // WAVE):
                        st = st_pool.tile([MB, WAVE, NT], F32)
                        for j in range(WAVE):
                            rg = 64 * j
                            nc.tensor.matmul(
                                st[:, j, :],
                                kt[w // 4][rg : rg + D, (w % 4) * MB : (w % 4 + 1) * MB],
                                qt[nt][rg : rg + D, :],
                                start=True,
                                stop=True,
                            )
                        e_t = e_pool.tile([MB, WAVE, NT], FP8)
                        nc.scalar.activation(e_t[:], st[:], AF.Exp, bias=ebias_sb[:])
                        pend.append((e_t, w))
                        # lag-2 in steady state, drained to lag-1 at the tile
                        # end so the boundary tail is one AV matmul, not two
                        while len(pend) > (2 if w < N_MB // WAVE - 1 else 1):
                            emit_av(*pend.pop(0))
                        # normalize of the PREVIOUS tile, mid-stream so the
                        # PE/DVE chain never gates a tile boundary. Wave 6:
                        # by then the 3.3us reciprocal is done, so the bc
                        # matmul never blocks the in-order PE queue.
                        if w == 6 and prev is not None:
                            norm_apply(*prev)
                            prev = None
                    for p in pend:
                        emit_av(*p)
                    prev = (av, norm_recip(av), nt)

                norm_apply(*prev)
    nc.compile()
    return nc


_NC = None


def _get_nc() -> bass.Bass:
    global _NC
    if _NC is None:
        _NC = build_program()
    return _NC


def make_in_maps(x_opt, x_sar, wq, bq, wk, bk, wv, bv, gamma):
    f = np.float32
    x_opt = np.asarray(x_opt, f).reshape(B, C, N)
    x_sar = np.asarray(x_sar, f).reshape(B, C, N)
    g = float(np.asarray(gamma, f).reshape(-1)[0])

    # weights without bias rows (biases are added in the PSUM->SBUF casts);
    # gamma folded into wv's v columns, denominator column stays 1
    wv_g = np.zeros((C, VW), f)
    wv_g[:, :C] = np.asarray(wv, f).T * g
    whalf = np.concatenate(
        [np.asarray(wq, f).T, np.asarray(wk, f).T, wv_g], axis=1
    )  # [64, WCOLS]
    w2 = np.ascontiguousarray(np.concatenate([whalf, whalf], 0).astype(NP_BF16))
    bpack = np.zeros((64 + D, 2), f)
    bpack[0:D, 0] = bpack[64 : 64 + D, 0] = np.asarray(bq, f)
    bpack[0:D, 1] = bpack[64 : 64 + D, 1] = np.asarray(bk, f)
    bvg_row = np.zeros((VW,), f)
    bvg_row[:C] = np.asarray(bv, f) * g
    bvg_row[C] = 1.0  # denominator column
    bvg4 = np.ascontiguousarray(np.tile(bvg_row, (MB, 4)))

    maps = []
    for core in range(NCORES):
        b, h = divmod(core, 2)
        xo = x_opt[b, :, h * NL : (h + 1) * NL]
        # stacked halves: partitions 0-63 = first half of the free axis,
        # partitions 64-127 = second half
        xo_st = np.ascontiguousarray(
            np.concatenate([xo[:, : NL // 2], xo[:, NL // 2 :]], 0).astype(NP_BF16)
        )
        xs = x_sar[b]
        xs_st = np.ascontiguousarray(
            np.concatenate([xs[:, : N // 2], xs[:, N // 2 :]], 0).astype(NP_BF16)
        )
        maps.append(
            {
                "xo_st": xo_st,
                "xs_st": xs_st,
                "xof": np.ascontiguousarray(xo),
                "w2": w2,
                "bpack": bpack,
                "bvg4": bvg4,
            }
        )
    return maps


def assemble_out(results) -> np.ndarray:
    out = np.empty((B, C, N), np.float32)
    for core in range(NCORES):
        b, h = divmod(core, 2)
        out[b, :, h * NL : (h + 1) * NL] = results[core]["out"]
    return out.reshape(B, C, HH, WW)


def kernel(**inputs) -> np.ndarray:
    nc = _get_nc()
    maps = make_in_maps(**inputs)
    res = run_bass_kernel_spmd(nc, maps, list(range(NCORES)))
    return assemble_out(res.results)


# revision 64
# speedup vs baseline: 1.0328x; 1.0328x over previous
"""Trainium2 Bass kernel for CrossModalFusion (B=4, C=64, H=W=64, N=4096).

Reference computation (per sample b, with x reshaped to [C, N]):
    q = wq @ xo + bq          [8, N]
    k = wk @ xs + bk          [8, N]
    v = wv @ xs + bv          [64, N]
    S[n, m]  = q[:, n] . k[:, m]
    attn     = softmax_m(S)
    out      = gamma * (v @ attn^T) + x_opt

Sharding: 8 cores = 4 batch samples x 2 halves of the query (n) axis.
Each core computes output rows [64, 2048] for its (sample, n-half); no
cross-core communication is needed.

Per-core dataflow:
  - TRN2 PE observation this kernel is built around: a single matmul
    instruction streams moving columns at ~1.2 Gcol/s regardless of dtype,
    but TWO matmuls in disjoint row tiles run concurrently (~2x), and fp8
    DoubleRow processes 2 contraction k-tiles per pass (~1.5x).
  - stacked-halves layout: inputs arrive with the free axis split across
    partitions 0-63 / 64-127, so every q/k/v projection matmul has a
    concurrent partner in the opposite row tile, and k lands at both row
    groups without replication (k rows 0-7 = m 0-2047, rows 64-71 =
    m 2048-4095).  Biases are added in the PSUM->SBUF casts (split across
    the ACT and DVE engines); gamma is folded into wv on the host (the
    denominator column stays 1, so gamma cancels in the divide).
  - scores are computed TRANSPOSED (S^T[m, n]) in bf16: wave w scores
    m-block w (rows 0-7) concurrently with m-block 16+w (rows 64-71) into
    one fp32 PSUM tile; exp runs on ACT reading PSUM and writing fp8e4
    SBUF directly (bias -1.5 keeps e^s inside fp8 range; it cancels in
    the softmax ratio).  No max-subtraction is needed: scores are O(3).
  - the attention*V accumulation is ONE fp8 DoubleRow matmul per wave
    (k-tile pair = blocks w and 16+w, stationary v^T padded to 80 cols so
    the k-tile step is 16-aligned).  v^T's column 64 is the all-ones
    denominator column, so sum_m exp(S[n, m]) accumulates for free.
  - the wave loop runs at the ACT exp floor (~1.11us per 2 m-blocks);
    AV lags two waves (one at tile end) so the PE never waits on exp.
  - per n-tile of 512: normalize via reciprocal_approx_fast (custom DVE
    op, needs an SBUF source), a rank-1 ones broadcast matmul, and two
    DVE element-wise ops.  The apply for tile t is emitted at wave 6 of
    tile t+1, after its reciprocal has finished, so the in-order PE queue
    never blocks on it.
  - residual x_opt is DMA'd separately in fp32 (off the critical path) so
    the gamma=0 output is exact (attention term is exactly zero).
"""

import os
import sys

import numpy as np

for _p in ("/opt/trn_rl_repo", "/root/.axon_site/_ro/trn_rl_repo"):
    if os.path.isdir(_p) and _p not in sys.path:
        sys.path.insert(0, _p)

import ml_dtypes

import concourse.bass as bass
import concourse.mybir as mybir
import concourse.tile as tile
from concourse import bacc
from concourse.bass_utils import run_bass_kernel_spmd

F32 = mybir.dt.float32
F32R = mybir.dt.float32r
BF16 = mybir.dt.bfloat16
FP8 = mybir.dt.float8e4
AF = mybir.ActivationFunctionType
NP_BF16 = np.dtype(ml_dtypes.bfloat16)
EXP_BIAS = -1.5  # exp(s + EXP_BIAS): keeps e_t within fp8e4 range; the
                 # common factor cancels between numerator and denominator

B, C, HH, WW = 4, 64, 64, 64
N = HH * WW            # 4096 key/query positions
D = 8                  # q/k channel count
CA = C + 1             # augmented channel dim (ones row / denominator row)
VW = 80                # padded v^T block width (DoubleRow k-tile step %16==0)
WCOLS = D + D + VW     # packed weight buffer width (wq | wk | wv')
NCORES = 8
NL = N // 2            # query rows per core
NT = 512               # n-tile (PSUM bank width in fp32)
MB = 128               # m-block (PE partition width)
N_NT = NL // NT        # 4 n-tiles per core
N_MB = N // MB         # 32 m-blocks
WAVE = 2               # m-blocks exp'd per ACT instruction


def build_program(repeat: int = 1) -> bass.Bass:
    # Bacc (not raw Bass): its compile() pass splits multi-semaphore waits
    # and moves matmul waits onto LDWEIGHTS, which this walrus build requires.
    nc = bacc.Bacc("TRN2", target_bir_lowering=False, num_devices=NCORES)
    # stacked-halves layout: partitions 0-63 hold one half of the data,
    # 64-127 the other, so every preamble matmul has a concurrent partner
    # in the opposite PE row tile (2x aggregate column throughput).
    xo_d = nc.declare_dram_parameter("xo_st", [MB, NL // 2], BF16, isOutput=False)
    xs_d = nc.declare_dram_parameter("xs_st", [MB, N // 2], BF16, isOutput=False)
    xof_d = nc.declare_dram_parameter("xof", [C, NL], F32, isOutput=False)
    w_d = nc.declare_dram_parameter("w2", [MB, WCOLS], BF16, isOutput=False)
    bp_d = nc.declare_dram_parameter("bpack", [64 + D, 2], F32, isOutput=False)
    bv_d = nc.declare_dram_parameter("bvg4", [MB, 4 * VW], F32, isOutput=False)
    out_d = nc.declare_dram_parameter("out", [C, NL], F32, isOutput=True)

    with tile.TileContext(nc) as tc:
      for _rep in range(repeat):
        with tc.tile_pool(name="const", bufs=1) as cp:
            # --- input DMAs, spread across per-engine DGE queues ---
            w_sb = cp.tile([MB, WCOLS], BF16)
            nc.gpsimd.dma_start(w_sb[:], w_d[:])
            xo_sb = cp.tile([MB, NL // 2], BF16)
            nc.sync.dma_start(xo_sb[:], xo_d[:])
            xs_sb = cp.tile([MB, N // 2], BF16)

            def load_xs(j):
                nc.gpsimd.dma_start(
                    xs_sb[:, j * 1024 : (j + 1) * 1024],
                    xs_d[:, j * 1024 : (j + 1) * 1024],
                )

            load_xs(0)
            bp_sb = cp.tile([64 + D, 2], F32)
            nc.gpsimd.dma_start(bp_sb[:], bp_d[:])
            bvg_sb = cp.tile([MB, 4 * VW], F32)
            nc.gpsimd.dma_start(bvg_sb[:], bv_d[:])
            xof_sb = cp.tile([C, NL], F32)
            ones_sb = cp.tile([1, C], BF16)
            nc.vector.memset(ones_sb[:], 1.0)
            ebias_sb = cp.tile([MB, 1], F32)
            nc.vector.memset(ebias_sb[:], EXP_BIAS)
            wqA, wqB = w_sb[0:64, 0:D], w_sb[64:MB, 0:D]
            wkA, wkB = w_sb[0:64, D : 2 * D], w_sb[64:MB, D : 2 * D]
            wvA, wvB = w_sb[0:64, 2 * D :], w_sb[64:MB, 2 * D :]

            # q holds all n at both partition offsets (row groups for the
            # rank-8 S^T matmuls); k rows 0-7 = m 0-2047, rows 64-71 =
            # m 2048-4095 (no replication needed)
            q_rep = cp.tile([64 + D, NL], BF16)
            k_rep = cp.tile([64 + D, NL], BF16)
            # v^T blocks in fp8e4: feeds the DoubleRow attention*V matmul
            vT = cp.tile([MB, N_MB, VW], FP8)

            with tc.tile_pool(name="pre_ps", bufs=2, space="PSUM") as pp:
                def emit_q(j):
                    # halves A/B run concurrently in row tiles 0/64
                    qp = pp.tile([MB, NT], F32, tag="qk_ps")
                    nc.tensor.matmul(
                        qp[0:D, :], wqA, xo_sb[0:64, j * NT : (j + 1) * NT],
                        start=True, stop=True,
                    )
                    nc.tensor.matmul(
                        qp[64 : 64 + D, :], wqB,
                        xo_sb[64:MB, j * NT : (j + 1) * NT],
                        start=True, stop=True,
                    )
                    slA = q_rep[0:D, j * NT : (j + 1) * NT]
                    slB = q_rep[64 : 64 + D, 1024 + j * NT : 1024 + (j + 1) * NT]
                    # bias add fused into the PSUM->bf16 casts (ACT is idle
                    # in the preamble)
                    nc.scalar.activation(
                        slA, qp[0:D, :], AF.Identity, bias=bp_sb[0:D, 0:1]
                    )
                    nc.scalar.activation(
                        slB, qp[64 : 64 + D, :], AF.Identity,
                        bias=bp_sb[64 : 64 + D, 0:1],
                    )
                    # partition-offset replicas: engines are lane-aligned,
                    # so these must be DMAs
                    nc.gpsimd.dma_start(
                        q_rep[64 : 64 + D, j * NT : (j + 1) * NT], slA
                    )
                    nc.gpsimd.dma_start(
                        q_rep[0:D, 1024 + j * NT : 1024 + (j + 1) * NT], slB
                    )

                def emit_k(j):
                    kp = pp.tile([MB, NT], F32, tag="qk_ps")
                    nc.tensor.matmul(
                        kp[0:D, :], wkA, xs_sb[0:64, j * NT : (j + 1) * NT],
                        start=True, stop=True,
                    )
                    nc.tensor.matmul(
                        kp[64 : 64 + D, :], wkB,
                        xs_sb[64:MB, j * NT : (j + 1) * NT],
                        start=True, stop=True,
                    )
                    # halve the serial cast chain: A-half on DVE, B-half on
                    # the (preamble-idle) ACT engine
                    nc.vector.tensor_scalar_add(
                        k_rep[0:D, j * NT : (j + 1) * NT], kp[0:D, :],
                        bp_sb[0:D, 1:2],
                    )
                    nc.scalar.activation(
                        k_rep[64 : 64 + D, j * NT : (j + 1) * NT],
                        kp[64 : 64 + D, :], AF.Identity,
                        bias=bp_sb[64 : 64 + D, 1:2],
                    )

                def emit_v(g):
                    # 4 m-block pairs per call; halves A/B concurrent; the
                    # bias row (bv*gamma | denominator 1) is added in the
                    # PSUM->fp8 cast
                    vpA = pp.tile([MB, 4 * VW], F32, tag="vpa_ps")
                    vpB = pp.tile([MB, 4 * VW], F32, tag="vpb_ps")
                    for t in range(4):
                        a = 4 * g + t
                        nc.tensor.matmul(
                            vpA[:, t * VW : (t + 1) * VW],
                            xs_sb[0:64, a * MB : (a + 1) * MB], wvA,
                            start=True, stop=True,
                        )
                        nc.tensor.matmul(
                            vpB[:, t * VW : (t + 1) * VW],
                            xs_sb[64:MB, a * MB : (a + 1) * MB], wvB,
                            start=True, stop=True,
                        )
                    nc.vector.tensor_add(
                        vT[:, 4 * g : 4 * (g + 1), :], vpA[:], bvg_sb[:]
                    )
                    nc.vector.tensor_add(
                        vT[:, 16 + 4 * g : 16 + 4 * (g + 1), :], vpB[:],
                        bvg_sb[:],
                    )

                # ordered by first use in the wave loop
                emit_q(0); emit_k(0); emit_v(0)
                load_xs(1)
                emit_q(1); emit_k(1); emit_v(1)
                emit_k(2); emit_v(2)
                emit_k(3); emit_v(3)

            # residual; needed late (first normalize), queued after replicas
            for j in range(2):
                nc.gpsimd.dma_start(
                    xof_sb[:, j * 1024 : (j + 1) * 1024],
                    xof_d[:, j * 1024 : (j + 1) * 1024],
                )

            with (
                tc.tile_pool(name="st_ps", bufs=2, space="PSUM") as st_pool,
                tc.tile_pool(name="av_ps", bufs=2, space="PSUM") as av_pool,
                tc.tile_pool(name="bc_ps", bufs=1, space="PSUM") as bc_pool,
                tc.tile_pool(name="e_sb", bufs=6) as e_pool,
                tc.tile_pool(name="o_sb", bufs=2) as o_pool,
                tc.tile_pool(name="sm_sb", bufs=2) as sm_pool,
            ):
                def norm_recip(av):
                    # softmax denominator -> reciprocal (row 64 of av).
                    # reciprocal_approx_fast (custom DVE op, ~51 ULP) is ~5x
                    # faster than the iterative divide; it MUST read SBUF (a
                    # PSUM source silently computes garbage), so stage the
                    # denominator row out of PSUM first.
                    ds = sm_pool.tile([1, NT], F32, tag="ds")
                    nc.vector.tensor_copy(ds[:], av[C:CA, :])
                    r = sm_pool.tile([1, NT], F32, tag="r")
                    nc.vector.reciprocal_approx_fast(out=r[:], in_=ds[:])
                    # bf16 copy feeds the broadcast matmul (bf16 is ~2x
                    # faster than the fp32 LOW/HIGH pair on the drain path)
                    rb = sm_pool.tile([1, NT], BF16, tag="rb")
                    nc.vector.tensor_copy(rb[:], r[:])
                    return rb

                def norm_apply(av, rb, nt):
                    n0, n1 = nt * NT, (nt + 1) * NT
                    # broadcast 1/denom across the 64 channel partitions via
                    # a rank-1 ones matmul
                    bc = bc_pool.tile([C, NT], F32)
                    nc.tensor.matmul(bc[:], ones_sb[:], rb[:], start=True, stop=True)
                    bcs = o_pool.tile([C, NT], F32, tag="bcs")
                    nc.vector.tensor_copy(bcs[:], bc[:])
                    om = o_pool.tile([C, NT], F32, tag="om")
                    nc.vector.tensor_mul(om[:], av[0:C, :], bcs[:])
                    o = o_pool.tile([C, NT], F32, tag="o")
                    nc.vector.tensor_add(o[:], om[:], xof_sb[:, n0:n1])
                    nc.sync.dma_start(out_d[:, n0:n1], o[:])

                prev = None  # (av, r, nt) awaiting apply
                for nt in range(N_NT):
                    n0, n1 = nt * NT, (nt + 1) * NT
                    av = av_pool.tile([CA, NT], F32)

                    def emit_av(e_t, w, av=av):
                        # fp8 DoubleRow: one matmul accumulates BOTH m-blocks
                        # of the wave (k-tile pair = blocks w and 16+w, so
                        # the 3D stationary AP strides 16 blocks apart)
                        a = vT[:, w : w + 2, 0:CA]
                        lhsT = bass.AP(
                            tensor=a.tensor, offset=a.offset,
                            ap=[list(a.ap[0]), [16 * VW, 2], [1, CA]],
                        )
                        nc.tensor.matmul(
                            av[:],
                            lhsT,
                            e_t[:],
                            perf_mode=mybir.MatmulPerfMode.DoubleRow,
                            start=(w == 0),
                            stop=(w == N_MB // WAVE - 1),
                        )

                    # S^T matmuls + exp, with the AV accumulation lagging two
                    # waves so the PE never stalls waiting on the current exp.
                    # Wave w scores m-block w (rows 0-7) and m-block 16+w
                    # (rows 64-71) -- both live at column w*128 of k_rep.
                    pend = []
                    for w in range(N_MB // WAVE):
                        st = st_pool.tile([MB, WAVE, NT], F32)
                        for j in range(WAVE):
                            rg = 64 * j
                            nc.tensor.matmul(
                                st[:, j, :],
                                k_rep[rg : rg + D, w * MB : (w + 1) * MB],
                                q_rep[rg : rg + D, n0:n1],
                                start=True,
                                stop=True,
                            )
                        e_t = e_pool.tile([MB, WAVE, NT], FP8)
                        nc.scalar.activation(e_t[:], st[:], AF.Exp, bias=ebias_sb[:])
                        pend.append((e_t, w))
                        # lag-2 in steady state, drained to lag-1 at the tile
                        # end so the boundary tail is one AV matmul, not two
                        while len(pend) > (2 if w < N_MB // WAVE - 1 else 1):
                            emit_av(*pend.pop(0))
                        # normalize of the PREVIOUS tile, mid-stream so the
                        # PE/DVE chain never gates a tile boundary. Wave 6:
                        # by then the 3.3us reciprocal is done, so the bc
                        # matmul never blocks the in-order PE queue.
                        if w == 6 and prev is not None:
                            norm_apply(*prev)
                            prev = None
                    for p in pend:
                        emit_av(*p)
                    prev = (av, norm_recip(av), nt)

                norm_apply(*prev)
    nc.compile()
    return nc


_NC = None


def _get_nc() -> bass.Bass:
    global _NC
    if _NC is None:
        _NC = build_program()
    return _NC


def make_in_maps(x_opt, x_sar, wq, bq, wk, bk, wv, bv, gamma):
    f = np.float32
    x_opt = np.asarray(x_opt, f).reshape(B, C, N)
    x_sar = np.asarray(x_sar, f).reshape(B, C, N)
    g = float(np.asarray(gamma, f).reshape(-1)[0])

    # weights without bias rows (biases are added in the PSUM->SBUF casts);
    # gamma folded into wv's v columns, denominator column stays 1
    wv_g = np.zeros((C, VW), f)
    wv_g[:, :C] = np.asarray(wv, f).T * g
    whalf = np.concatenate(
        [np.asarray(wq, f).T, np.asarray(wk, f).T, wv_g], axis=1
    )  # [64, WCOLS]
    w2 = np.ascontiguousarray(np.concatenate([whalf, whalf], 0).astype(NP_BF16))
    bpack = np.zeros((64 + D, 2), f)
    bpack[0:D, 0] = bpack[64 : 64 + D, 0] = np.asarray(bq, f)
    bpack[0:D, 1] = bpack[64 : 64 + D, 1] = np.asarray(bk, f)
    bvg_row = np.zeros((VW,), f)
    bvg_row[:C] = np.asarray(bv, f) * g
    bvg_row[C] = 1.0  # denominator column
    bvg4 = np.ascontiguousarray(np.tile(bvg_row, (MB, 4)))

    maps = []
    for core in range(NCORES):
        b, h = divmod(core, 2)
        xo = x_opt[b, :, h * NL : (h + 1) * NL]
        # stacked halves: partitions 0-63 = first half of the free axis,
        # partitions 64-127 = second half
        xo_st = np.ascontiguousarray(
            np.concatenate([xo[:, : NL // 2], xo[:, NL // 2 :]], 0).astype(NP_BF16)
        )
        xs = x_sar[b]
        xs_st = np.ascontiguousarray(
            np.concatenate([xs[:, : N // 2], xs[:, N // 2 :]], 0).astype(NP_BF16)
        )
        maps.append(
            {
                "xo_st": xo_st,
                "xs_st": xs_st,
                "xof": np.ascontiguousarray(xo),
                "w2": w2,
                "bpack": bpack,
                "bvg4": bvg4,
            }
        )
    return maps


def assemble_out(results) -> np.ndarray:
    out = np.empty((B, C, N), np.float32)
    for core in range(NCORES):
        b, h = divmod(core, 2)
        out[b, :, h * NL : (h + 1) * NL] = results[core]["out"]
    return out.reshape(B, C, HH, WW)


def kernel(**inputs) -> np.ndarray:
    nc = _get_nc()
    maps = make_in_maps(**inputs)
    res = run_bass_kernel_spmd(nc, maps, list(range(NCORES)))
    return assemble_out(res.results)


# revision 65
# speedup vs baseline: 1.0474x; 1.0141x over previous
"""Trainium2 Bass kernel for CrossModalFusion (B=4, C=64, H=W=64, N=4096).

Reference computation (per sample b, with x reshaped to [C, N]):
    q = wq @ xo + bq          [8, N]
    k = wk @ xs + bk          [8, N]
    v = wv @ xs + bv          [64, N]
    S[n, m]  = q[:, n] . k[:, m]
    attn     = softmax_m(S)
    out      = gamma * (v @ attn^T) + x_opt

Sharding: 8 cores = 4 batch samples x 2 halves of the query (n) axis.
Each core computes output rows [64, 2048] for its (sample, n-half); no
cross-core communication is needed.

Per-core dataflow:
  - TRN2 PE observation this kernel is built around: a single matmul
    instruction streams moving columns at ~1.2 Gcol/s regardless of dtype,
    but TWO matmuls in disjoint row tiles run concurrently (~2x), and fp8
    DoubleRow processes 2 contraction k-tiles per pass (~1.5x).
  - stacked-halves layout: inputs arrive with the free axis split across
    partitions 0-63 / 64-127, so every q/k/v projection matmul has a
    concurrent partner in the opposite row tile, and k lands at both row
    groups without replication (k rows 0-7 = m 0-2047, rows 64-71 =
    m 2048-4095).  Biases are added in the PSUM->SBUF casts (split across
    the ACT and DVE engines); gamma is folded into wv on the host (the
    denominator column stays 1, so gamma cancels in the divide).
  - scores are computed TRANSPOSED (S^T[m, n]) in bf16: wave w scores
    m-block w (rows 0-7) concurrently with m-block 16+w (rows 64-71) into
    one fp32 PSUM tile; exp runs on ACT reading PSUM and writing fp8e4
    SBUF directly (bias -1.5 keeps e^s inside fp8 range; it cancels in
    the softmax ratio).  No max-subtraction is needed: scores are O(3).
  - the attention*V accumulation is ONE fp8 DoubleRow matmul per wave
    (k-tile pair = blocks w and 16+w, stationary v^T padded to 80 cols so
    the k-tile step is 16-aligned).  v^T's column 64 is the all-ones
    denominator column, so sum_m exp(S[n, m]) accumulates for free.
  - the wave loop runs at the ACT exp floor (~1.11us per 2 m-blocks);
    AV lags two waves (one at tile end) so the PE never waits on exp.
  - per n-tile of 512: normalize via reciprocal_approx_fast (custom DVE
    op, needs an SBUF source), a rank-1 ones broadcast matmul, and two
    DVE element-wise ops.  The apply for tile t is emitted at wave 6 of
    tile t+1, after its reciprocal has finished, so the in-order PE queue
    never blocks on it.
  - residual x_opt is DMA'd separately in fp32 (off the critical path) so
    the gamma=0 output is exact (attention term is exactly zero).
"""

import os
import sys

import numpy as np

for _p in ("/opt/trn_rl_repo", "/root/.axon_site/_ro/trn_rl_repo"):
    if os.path.isdir(_p) and _p not in sys.path:
        sys.path.insert(0, _p)

import ml_dtypes

import concourse.bass as bass
import concourse.mybir as mybir
import concourse.tile as tile
from concourse import bacc
from concourse.bass_utils import run_bass_kernel_spmd

F32 = mybir.dt.float32
F32R = mybir.dt.float32r
BF16 = mybir.dt.bfloat16
FP8 = mybir.dt.float8e4
AF = mybir.ActivationFunctionType
NP_BF16 = np.dtype(ml_dtypes.bfloat16)
EXP_BIAS = -1.5  # exp(s + EXP_BIAS): keeps e_t within fp8e4 range; the
                 # common factor cancels between numerator and denominator

B, C, HH, WW = 4, 64, 64, 64
N = HH * WW            # 4096 key/query positions
D = 8                  # q/k channel count
CA = C + 1             # augmented channel dim (ones row / denominator row)
VW = 80                # padded v^T block width (DoubleRow k-tile step %16==0)
WCOLS = D + D + VW     # packed weight buffer width (wq | wk | wv')
NCORES = 8
NL = N // 2            # query rows per core
NT = 512               # n-tile (PSUM bank width in fp32)
MB = 128               # m-block (PE partition width)
N_NT = NL // NT        # 4 n-tiles per core
N_MB = N // MB         # 32 m-blocks
WAVE = 2               # m-blocks exp'd per ACT instruction


def build_program(repeat: int = 1) -> bass.Bass:
    # Bacc (not raw Bass): its compile() pass splits multi-semaphore waits
    # and moves matmul waits onto LDWEIGHTS, which this walrus build requires.
    nc = bacc.Bacc("TRN2", target_bir_lowering=False, num_devices=NCORES)
    # stacked-halves layout: partitions 0-63 hold one half of the data,
    # 64-127 the other, so every preamble matmul has a concurrent partner
    # in the opposite PE row tile (2x aggregate column throughput).
    xo_d = nc.declare_dram_parameter("xo_st", [MB, NL // 2], BF16, isOutput=False)
    xs_d = nc.declare_dram_parameter("xs_st", [MB, N // 2], BF16, isOutput=False)
    xof_d = nc.declare_dram_parameter("xof", [C, NL], F32, isOutput=False)
    w_d = nc.declare_dram_parameter("w2", [MB, WCOLS], BF16, isOutput=False)
    bp_d = nc.declare_dram_parameter("bpack", [64 + D, 2], F32, isOutput=False)
    bv_d = nc.declare_dram_parameter("bvg4", [MB, 4 * VW], F32, isOutput=False)
    out_d = nc.declare_dram_parameter("out", [C, NL], F32, isOutput=True)

    with tile.TileContext(nc) as tc:
      for _rep in range(repeat):
        with tc.tile_pool(name="const", bufs=1) as cp:
            # --- input DMAs, spread across per-engine DGE queues ---
            w_sb = cp.tile([MB, WCOLS], BF16)
            nc.gpsimd.dma_start(w_sb[:], w_d[:])
            xo_sb = cp.tile([MB, NL // 2], BF16)
            nc.sync.dma_start(xo_sb[:], xo_d[:])
            xs_sb = cp.tile([MB, N // 2], BF16)

            def load_xs(j):
                nc.gpsimd.dma_start(
                    xs_sb[:, j * 1024 : (j + 1) * 1024],
                    xs_d[:, j * 1024 : (j + 1) * 1024],
                )

            load_xs(0)
            bp_sb = cp.tile([64 + D, 2], F32)
            nc.gpsimd.dma_start(bp_sb[:], bp_d[:])
            bvg_sb = cp.tile([MB, 4 * VW], F32)
            nc.gpsimd.dma_start(bvg_sb[:], bv_d[:])
            xof_sb = cp.tile([C, NL], F32)
            ones_sb = cp.tile([1, C], BF16)
            nc.vector.memset(ones_sb[:], 1.0)
            ebias_sb = cp.tile([MB, 1], F32)
            nc.vector.memset(ebias_sb[:], EXP_BIAS)
            wqA, wqB = w_sb[0:64, 0:D], w_sb[64:MB, 0:D]
            wkA, wkB = w_sb[0:64, D : 2 * D], w_sb[64:MB, D : 2 * D]
            wvA, wvB = w_sb[0:64, 2 * D :], w_sb[64:MB, 2 * D :]

            # q holds all n at both partition offsets (row groups for the
            # rank-8 S^T matmuls); k rows 0-7 = m 0-2047, rows 64-71 =
            # m 2048-4095 (no replication needed)
            q_rep = cp.tile([64 + D, NL], BF16)
            k_rep = cp.tile([64 + D, NL], BF16)
            # v^T blocks in fp8e4: feeds the DoubleRow attention*V matmul
            vT = cp.tile([MB, N_MB, VW], FP8)

            with (
                tc.tile_pool(name="qk_ps", bufs=4, space="PSUM") as qkp,
                tc.tile_pool(name="pre_ps", bufs=2, space="PSUM") as pp,
            ):
                def emit_q(j):
                    # halves A/B run concurrently in row tiles 0/64
                    qp = qkp.tile([MB, NT], F32, tag="qk_ps")
                    nc.tensor.matmul(
                        qp[0:D, :], wqA, xo_sb[0:64, j * NT : (j + 1) * NT],
                        start=True, stop=True,
                    )
                    nc.tensor.matmul(
                        qp[64 : 64 + D, :], wqB,
                        xo_sb[64:MB, j * NT : (j + 1) * NT],
                        start=True, stop=True,
                    )
                    slA = q_rep[0:D, j * NT : (j + 1) * NT]
                    slB = q_rep[64 : 64 + D, 1024 + j * NT : 1024 + (j + 1) * NT]
                    # bias add fused into the PSUM->bf16 casts (ACT is idle
                    # in the preamble)
                    nc.scalar.activation(
                        slA, qp[0:D, :], AF.Identity, bias=bp_sb[0:D, 0:1]
                    )
                    nc.scalar.activation(
                        slB, qp[64 : 64 + D, :], AF.Identity,
                        bias=bp_sb[64 : 64 + D, 0:1],
                    )
                    # partition-offset replicas: engines are lane-aligned,
                    # so these must be DMAs
                    nc.gpsimd.dma_start(
                        q_rep[64 : 64 + D, j * NT : (j + 1) * NT], slA
                    )
                    nc.gpsimd.dma_start(
                        q_rep[0:D, 1024 + j * NT : 1024 + (j + 1) * NT], slB
                    )

                def emit_k(j):
                    kp = qkp.tile([MB, NT], F32, tag="qk_ps")
                    nc.tensor.matmul(
                        kp[0:D, :], wkA, xs_sb[0:64, j * NT : (j + 1) * NT],
                        start=True, stop=True,
                    )
                    nc.tensor.matmul(
                        kp[64 : 64 + D, :], wkB,
                        xs_sb[64:MB, j * NT : (j + 1) * NT],
                        start=True, stop=True,
                    )
                    # halve the serial cast chain: A-half on DVE, B-half on
                    # the (preamble-idle) ACT engine
                    nc.vector.tensor_scalar_add(
                        k_rep[0:D, j * NT : (j + 1) * NT], kp[0:D, :],
                        bp_sb[0:D, 1:2],
                    )
                    nc.scalar.activation(
                        k_rep[64 : 64 + D, j * NT : (j + 1) * NT],
                        kp[64 : 64 + D, :], AF.Identity,
                        bias=bp_sb[64 : 64 + D, 1:2],
                    )

                def emit_v(g):
                    # 4 m-block pairs per call; halves A/B concurrent; the
                    # bias row (bv*gamma | denominator 1) is added in the
                    # PSUM->fp8 cast
                    vpA = pp.tile([MB, 4 * VW], F32, tag="vpa_ps")
                    vpB = pp.tile([MB, 4 * VW], F32, tag="vpb_ps")
                    for t in range(4):
                        a = 4 * g + t
                        nc.tensor.matmul(
                            vpA[:, t * VW : (t + 1) * VW],
                            xs_sb[0:64, a * MB : (a + 1) * MB], wvA,
                            start=True, stop=True,
                        )
                        nc.tensor.matmul(
                            vpB[:, t * VW : (t + 1) * VW],
                            xs_sb[64:MB, a * MB : (a + 1) * MB], wvB,
                            start=True, stop=True,
                        )
                    nc.vector.tensor_add(
                        vT[:, 4 * g : 4 * (g + 1), :], vpA[:], bvg_sb[:]
                    )
                    nc.vector.tensor_add(
                        vT[:, 16 + 4 * g : 16 + 4 * (g + 1), :], vpB[:],
                        bvg_sb[:],
                    )

                # ordered by first use in the wave loop
                emit_q(0); emit_k(0); emit_v(0)
                load_xs(1)
                emit_q(1); emit_k(1); emit_v(1)
                emit_k(2); emit_v(2)
                emit_k(3); emit_v(3)

            # residual; needed late (first normalize), queued after replicas
            for j in range(2):
                nc.gpsimd.dma_start(
                    xof_sb[:, j * 1024 : (j + 1) * 1024],
                    xof_d[:, j * 1024 : (j + 1) * 1024],
                )

            with (
                tc.tile_pool(name="st_ps", bufs=2, space="PSUM") as st_pool,
                tc.tile_pool(name="av_ps", bufs=2, space="PSUM") as av_pool,
                tc.tile_pool(name="bc_ps", bufs=1, space="PSUM") as bc_pool,
                tc.tile_pool(name="e_sb", bufs=6) as e_pool,
                tc.tile_pool(name="o_sb", bufs=2) as o_pool,
                tc.tile_pool(name="sm_sb", bufs=2) as sm_pool,
            ):
                def norm_recip(av):
                    # softmax denominator -> reciprocal (row 64 of av).
                    # reciprocal_approx_fast (custom DVE op, ~51 ULP) is ~5x
                    # faster than the iterative divide; it MUST read SBUF (a
                    # PSUM source silently computes garbage), so stage the
                    # denominator row out of PSUM first.
                    ds = sm_pool.tile([1, NT], F32, tag="ds")
                    nc.vector.tensor_copy(ds[:], av[C:CA, :])
                    r = sm_pool.tile([1, NT], F32, tag="r")
                    nc.vector.reciprocal_approx_fast(out=r[:], in_=ds[:])
                    # bf16 copy feeds the broadcast matmul (bf16 is ~2x
                    # faster than the fp32 LOW/HIGH pair on the drain path)
                    rb = sm_pool.tile([1, NT], BF16, tag="rb")
                    nc.vector.tensor_copy(rb[:], r[:])
                    return rb

                def norm_apply(av, rb, nt):
                    n0, n1 = nt * NT, (nt + 1) * NT
                    # broadcast 1/denom across the 64 channel partitions via
                    # a rank-1 ones matmul
                    bc = bc_pool.tile([C, NT], F32)
                    nc.tensor.matmul(bc[:], ones_sb[:], rb[:], start=True, stop=True)
                    bcs = o_pool.tile([C, NT], F32, tag="bcs")
                    nc.vector.tensor_copy(bcs[:], bc[:])
                    om = o_pool.tile([C, NT], F32, tag="om")
                    nc.vector.tensor_mul(om[:], av[0:C, :], bcs[:])
                    o = o_pool.tile([C, NT], F32, tag="o")
                    nc.vector.tensor_add(o[:], om[:], xof_sb[:, n0:n1])
                    nc.sync.dma_start(out_d[:, n0:n1], o[:])

                prev = None  # (av, r, nt) awaiting apply
                for nt in range(N_NT):
                    n0, n1 = nt * NT, (nt + 1) * NT
                    av = av_pool.tile([CA, NT], F32)

                    def emit_av(e_t, w, av=av):
                        # fp8 DoubleRow: one matmul accumulates BOTH m-blocks
                        # of the wave (k-tile pair = blocks w and 16+w, so
                        # the 3D stationary AP strides 16 blocks apart)
                        a = vT[:, w : w + 2, 0:CA]
                        lhsT = bass.AP(
                            tensor=a.tensor, offset=a.offset,
                            ap=[list(a.ap[0]), [16 * VW, 2], [1, CA]],
                        )
                        nc.tensor.matmul(
                            av[:],
                            lhsT,
                            e_t[:],
                            perf_mode=mybir.MatmulPerfMode.DoubleRow,
                            start=(w == 0),
                            stop=(w == N_MB // WAVE - 1),
                        )

                    # S^T matmuls + exp, with the AV accumulation lagging two
                    # waves so the PE never stalls waiting on the current exp.
                    # Wave w scores m-block w (rows 0-7) and m-block 16+w
                    # (rows 64-71) -- both live at column w*128 of k_rep.
                    pend = []
                    for w in range(N_MB // WAVE):
                        st = st_pool.tile([MB, WAVE, NT], F32)
                        for j in range(WAVE):
                            rg = 64 * j
                            nc.tensor.matmul(
                                st[:, j, :],
                                k_rep[rg : rg + D, w * MB : (w + 1) * MB],
                                q_rep[rg : rg + D, n0:n1],
                                start=True,
                                stop=True,
                            )
                        e_t = e_pool.tile([MB, WAVE, NT], FP8)
                        nc.scalar.activation(e_t[:], st[:], AF.Exp, bias=ebias_sb[:])
                        pend.append((e_t, w))
                        # lag-2 in steady state, drained to lag-1 at the tile
                        # end so the boundary tail is one AV matmul, not two
                        while len(pend) > (2 if w < N_MB // WAVE - 1 else 1):
                            emit_av(*pend.pop(0))
                        # normalize of the PREVIOUS tile, mid-stream so the
                        # PE/DVE chain never gates a tile boundary. Wave 6:
                        # by then the 3.3us reciprocal is done, so the bc
                        # matmul never blocks the in-order PE queue.
                        if w == 6 and prev is not None:
                            norm_apply(*prev)
                            prev = None
                    for p in pend:
                        emit_av(*p)
                    prev = (av, norm_recip(av), nt)

                norm_apply(*prev)
    nc.compile()
    return nc


_NC = None


def _get_nc() -> bass.Bass:
    global _NC
    if _NC is None:
        _NC = build_program()
    return _NC


def make_in_maps(x_opt, x_sar, wq, bq, wk, bk, wv, bv, gamma):
    f = np.float32
    x_opt = np.asarray(x_opt, f).reshape(B, C, N)
    x_sar = np.asarray(x_sar, f).reshape(B, C, N)
    g = float(np.asarray(gamma, f).reshape(-1)[0])

    # weights without bias rows (biases are added in the PSUM->SBUF casts);
    # gamma folded into wv's v columns, denominator column stays 1
    wv_g = np.zeros((C, VW), f)
    wv_g[:, :C] = np.asarray(wv, f).T * g
    whalf = np.concatenate(
        [np.asarray(wq, f).T, np.asarray(wk, f).T, wv_g], axis=1
    )  # [64, WCOLS]
    w2 = np.ascontiguousarray(np.concatenate([whalf, whalf], 0).astype(NP_BF16))
    bpack = np.zeros((64 + D, 2), f)
    bpack[0:D, 0] = bpack[64 : 64 + D, 0] = np.asarray(bq, f)
    bpack[0:D, 1] = bpack[64 : 64 + D, 1] = np.asarray(bk, f)
    bvg_row = np.zeros((VW,), f)
    bvg_row[:C] = np.asarray(bv, f) * g
    bvg_row[C] = 1.0  # denominator column
    bvg4 = np.ascontiguousarray(np.tile(bvg_row, (MB, 4)))

    maps = []
    for core in range(NCORES):
        b, h = divmod(core, 2)
        xo = x_opt[b, :, h * NL : (h + 1) * NL]
        # stacked halves: partitions 0-63 = first half of the free axis,
        # partitions 64-127 = second half
        xo_st = np.ascontiguousarray(
            np.concatenate([xo[:, : NL // 2], xo[:, NL // 2 :]], 0).astype(NP_BF16)
        )
        xs = x_sar[b]
        xs_st = np.ascontiguousarray(
            np.concatenate([xs[:, : N // 2], xs[:, N // 2 :]], 0).astype(NP_BF16)
        )
        maps.append(
            {
                "xo_st": xo_st,
                "xs_st": xs_st,
                "xof": np.ascontiguousarray(xo),
                "w2": w2,
                "bpack": bpack,
                "bvg4": bvg4,
            }
        )
    return maps


def assemble_out(results) -> np.ndarray:
    out = np.empty((B, C, N), np.float32)
    for core in range(NCORES):
        b, h = divmod(core, 2)
        out[b, :, h * NL : (h + 1) * NL] = results[core]["out"]
    return out.reshape(B, C, HH, WW)


def kernel(**inputs) -> np.ndarray:
    nc = _get_nc()
    maps = make_in_maps(**inputs)
    res = run_bass_kernel_spmd(nc, maps, list(range(NCORES)))
    return assemble_out(res.results)
